# revision 22
# baseline (speedup 1.0000x reference)
"""CrossSharedUnit Trainium2 kernel — 8-core data-parallel over batch.

Reference computation (per batch b, S=128 tokens, H=512 hidden, K=8):
  proj[b,s,k,g] = sum_h left[b,s,h] * G[h,k,g]
  raw[b,s,t,k]  = tanh(sum_g proj[b,s,k,g] * right[b,t,g])
  score[b,s,t]  = sum_k raw[b,s,t,k] * v[k]
  attn          = softmax(score, axis=t)
  out           = self + attn @ other_hidden
for two branches (aspect: left=aspect, right=polarity; polarity: left=aspect,
right=aspect — faithful to the source which uses aspect on BOTH sides).

Sharding: batch B=32 split 4-per-core across 8 cores; G tensors replicated.
No collectives.

Precision: all matmul operands are fp16 with fp32 PSUM accumulation — fp16's
10-bit mantissa matches the fp32r (TF32) datapath the fp32 version would use,
so accuracy is unchanged (measured ~2e-3 rel err, gate 2e-2) while every
input transfer halves and the DVE score chains run in 2x 16-bit mode.
exp() cannot overflow fp16: |score| <= sum|v_k| ~ 7.7 -> e^7.7 ~ 2.2e3.

Schedule: the PE (tensor engine) is the bottleneck (~76us of matmul), so the
program is one continuous PE stream:
  warmup | br0-s1 (+ br0-s2-ck0 spliced at k=5) | br0-s2-ck1
         | br1-s1 (+ br1-s2-ck0 splice + br0 z/out mms interleaved)
         | br1-s2-ck1 (pairwise, z/out + wm fillers interleaved)
with softmax chains on vector/scalar underneath the next phase's matmuls.

DMA: all input loads ride the sync queue in exact consumption order. The
startup-critical tensors (xa + G k0 block) are packed interleaved in two
dedicated head images so each dma_start covers >=4KB-contiguous partition
rows (small descriptors halve early DMA throughput), letting the first real
matmul start right as the warmup ramp ends.

The softmax division is deferred through the attention matmul:
out = self + (E @ other) / Z with Z from a ones-matmul.
"""

import os
import sys

sys.path.insert(0, "/opt/trn_rl_repo")

import numpy as np

from concourse import bacc, mybir, tile
from concourse.bass_utils import run_bass_kernel_spmd

B, S, H, K = 32, 128, 512, 8
NCORES = 8
BL = B // NCORES          # batches per core
BS = BL * S               # rows per core (512)
P = 128                   # partitions
HT = H // P               # h partition-tiles (4)
KG = K * H                # flattened (k,g) axis (4096)
KC = K // 2               # k's per stage-2 chunk (4)
F32 = mybir.dt.float32
F32R = mybir.dt.float32r
F16 = mybir.dt.float16

_cache = {}


def _build():
    """Build + compile the per-core Bass program (same program on all cores)."""
    nc = bacc.Bacc("TRN2", target_bir_lowering=False, debug=False,
                   num_devices=NCORES)

    # head images: [xa_h0|xa_h1|g_k0h0|g_k0h1] and the h2/h3 twin (fp16)
    head_a_d = nc.dram_tensor("head_a", [P, 2 * BS + 2 * H], F16,
                              kind="ExternalInput")
    head_b_d = nc.dram_tensor("head_b", [P, 2 * BS + 2 * H], F16,
                              kind="ExternalInput")
    xp_t_d = nc.dram_tensor("xp_t", [P, HT * BS], F16, kind="ExternalInput")
    xa_nat_d = nc.dram_tensor("xa_nat", [P, BL * H], F16, kind="ExternalInput")
    xp_nat_d = nc.dram_tensor("xp_nat", [P, BL * H], F16, kind="ExternalInput")
    # G: one pre-shuffled tensor per branch; column blocks in consumption
    # order: k0 h-major block (4H), k1 h-major block (4H), then per-k blocks
    g_ap_d = nc.dram_tensor("g_ap", [P, HT * KG], F16, kind="ExternalInput")
    g_pa_d = nc.dram_tensor("g_pa", [P, HT * KG], F16, kind="ExternalInput")
    v_ap_d = nc.dram_tensor("v_ap", [K, 1], F32, kind="ExternalInput")
    v_pa_d = nc.dram_tensor("v_pa", [K, 1], F32, kind="ExternalInput")
    out_a_d = nc.dram_tensor("out_a", [BS, H], F16, kind="ExternalOutput")
    out_p_d = nc.dram_tensor("out_p", [BS, H], F16, kind="ExternalOutput")

    Tanh = mybir.ActivationFunctionType.Tanh
    Exp = mybir.ActivationFunctionType.Exp
    MULT = mybir.AluOpType.mult
    ADD = mybir.AluOpType.add

    with tile.TileContext(nc) as tc:
        with (
            tc.tile_pool(name="const", bufs=1) as cpool,
            tc.tile_pool(name="g", bufs=1) as gpool,
            tc.tile_pool(name="proj", bufs=1) as projpool,
            tc.tile_pool(name="work", bufs=2) as work,
            tc.tile_pool(name="ps_acc", bufs=5, space="PSUM") as ps_acc,
            tc.tile_pool(name="ps_o", bufs=2, space="PSUM") as ps_o,
            tc.tile_pool(name="ps_z", bufs=1, space="PSUM") as ps_z,
        ):
            # ---- constants + warmup weights --------------------------------
            wm = cpool.tile([P, BS], F32R, tag="wm")
            nc.vector.memset(wm[:].bitcast(F32), 0.0)
            ones_t = cpool.tile([P, 2], F16, tag="ones_t")
            nc.vector.memset(ones_t[:], 1.0)

            # ---- persistent activations ------------------------------------
            head_a = cpool.tile([P, 2 * BS + 2 * H], F16, tag="head_a")
            head_b = cpool.tile([P, 2 * BS + 2 * H], F16, tag="head_b")
            xp_t = cpool.tile([P, HT * BS], F16, tag="xp_t")
            xa_nat = cpool.tile([P, BL * H], F16, tag="xa_nat")
            xp_nat = cpool.tile([P, BL * H], F16, tag="xp_nat")

            vrow_a = cpool.tile([1, K], F32, tag="vrow_a")
            vrow_p = cpool.tile([1, K], F32, tag="vrow_p")
            vbc_a = cpool.tile([P, K], F32, tag="vbc_a")
            vbc_p = cpool.tile([P, K], F32, tag="vbc_p")

            # ---- G piece tiles (bufs=1; k0/k1 recycled for branch 1) -------
            g_k0 = gpool.tile([P, HT * H], F16, tag="g_k0")
            g_k1 = gpool.tile([P, HT * H], F16, tag="g_k1")
            g_kk = [gpool.tile([P, HT * H], F16, tag=f"g_kk{i}",
                               name=f"g_kk{i}")
                    for i in range(6)]

            def load_g_k(g_d, i):
                o = (2 + i) * HT * H
                nc.sync.dma_start(out=g_kk[i][:], in_=g_d.ap()[:, o:o + HT * H])

            # ---- startup loads, consumption order. The first k1/k2 pieces
            # ride the gpsimd/scalar/vector queues so their issue+transfer
            # overlaps the sync queue's head stream (issues serialize ~1us
            # per dma_start within one queue).
            nc.sync.dma_start(out=head_a[:], in_=head_a_d.ap()[:])
            nc.sync.dma_start(out=head_b[:], in_=head_b_d.ap()[:])
            nc.gpsimd.dma_start(out=vrow_a[:], in_=v_ap_d.ap().rearrange("k o -> o k"))
            nc.gpsimd.dma_start(out=vrow_p[:], in_=v_pa_d.ap().rearrange("k o -> o k"))
            nc.gpsimd.partition_broadcast(vbc_a[:], vrow_a[:])
            nc.gpsimd.partition_broadcast(vbc_p[:], vrow_p[:])
            nc.sync.dma_start(out=g_k1[:], in_=g_ap_d.ap()[:, 4 * H:8 * H])
            for i in range(4):
                load_g_k(g_ap_d, i)
            nc.sync.dma_start(out=xp_t[:], in_=xp_t_d.ap()[:])
            load_g_k(g_ap_d, 4)
            load_g_k(g_ap_d, 5)
            nc.sync.dma_start(out=xp_nat[:], in_=xp_nat_d.ap()[:])
            # (g_pa loads are emitted after br0-s1 so the WAR deps pick up
            #  br0's reads; xa_nat after those.)

            def xa_rhs(h):
                t = head_a if h < 2 else head_b
                return t[:, (h % 2) * BS:(h % 2 + 1) * BS]

            def xa_lhsT(gi, b):
                t = head_a if gi < 2 else head_b
                o = (gi % 2) * BS + b * S
                return t[:, o:o + S]

            def xp_lhsT(gi, b):
                o = gi * BS + b * S
                return xp_t[:, o:o + S]

            def g0_head(h, gt):
                # branch-0 k0 weights live in the head images
                t = head_a if h < 2 else head_b
                o = 2 * BS + (h % 2) * H + gt * P
                return t[:, o:o + P]

            def g0_tile(h, gt):
                return g_k0[:, h * H + gt * P:h * H + gt * P + P]

            def g_lhsT(k, h, gt, g0_at):
                if k == 0:
                    return g0_at(h, gt)
                piece = g_k1 if k == 1 else g_kk[k - 2]
                o = h * H + gt * P
                return piece[:, o:o + P]

            # projT2[br][gt][g_part, k, b, s] — stage-1 output, stage-2 rhs;
            # per-branch buffers so br1's evacs never WAR-wait on br0-s2.
            projT2 = [[projpool.tile([P, K, BL, S], F16,
                                     tag=f"projT2_{br}_{gt}",
                                     name=f"projT2_{br}_{gt}")
                       for gt in range(HT)]
                      for br in range(2)]

            evac_state = [0]

            def evac(dst, src):
                # ping-pong PSUM evacuations between vector and scalar
                if evac_state[0] % 2 == 0:
                    nc.vector.tensor_copy(dst, src)
                else:
                    nc.scalar.copy(dst, src)
                evac_state[0] += 1

            # ---- PE warmup: get the p-state ramp going during DMA lead-in
            for w in range(7):
                acc = ps_acc.tile([P, BL, S], F32, tag="acc", name=f"warm{w}")
                nc.tensor.matmul(acc[:], wm[:, 0:P], wm[:],
                                 start=True, stop=True)

            def filler(name):
                acc = ps_acc.tile([P, BL, S], F32, tag="acc", name=name)
                nc.tensor.matmul(acc[:], wm[:, 0:P], wm[:],
                                 start=True, stop=True)

            def stage1(br, g0_at):
                # k0 h-outer with 4 open accumulators: first matmuls need
                # only the head_a image.
                accs = [ps_acc.tile([P, BL, S], F32, tag="acc",
                                    name=f"s1a{br}k0g{gt}")
                        for gt in range(HT)]
                for h in range(HT):
                    for gt in range(HT):
                        nc.tensor.matmul(
                            accs[gt][:], g_lhsT(0, h, gt, g0_at), xa_rhs(h),
                            start=(h == 0), stop=(h == HT - 1),
                            skip_group_check=True)
                for gt in range(HT):
                    evac(projT2[br][gt][:, 0, :, :], accs[gt][:])
                for k in range(1, K):
                    for gt in range(HT):
                        acc = ps_acc.tile([P, BL, S], F32, tag="acc",
                                          name=f"s1a{br}k{k}g{gt}")
                        for h in range(HT):
                            nc.tensor.matmul(
                                acc[:], g_lhsT(k, h, gt, g0_at), xa_rhs(h),
                                start=(h == 0), stop=(h == HT - 1))
                        evac(projT2[br][gt][:, k, :, :], acc[:])
                    yield k

            # th_all[t_part, k, b, s]: tanh(stage-2) output, both branches
            # (WAR-recycled). Score ops slice [:, j, :, :] batched over b.
            th_all = work.tile([P, K, BL, S], F16, tag="th", bufs=1)

            def stage2_ck(br, lhsT_of, klo, nk, bs=range(BL)):
                # raw[t, k, s] = tanh(sum_g right[t,g] proj[g,k,s]) per batch
                for b in bs:
                    acc2 = ps_acc.tile([P, nk, S], F32, tag="acc",
                                       name=f"s2a{br}b{b}c{klo}")
                    for gi in range(HT):
                        nc.tensor.matmul(
                            acc2[:],
                            lhsT_of(gi, b),
                            projT2[br][gi][:, klo:klo + nk, b, :],
                            start=(gi == 0), stop=(gi == HT - 1))
                    nc.scalar.activation(
                        th_all[:, klo:klo + nk, b, :], acc2[:], Tanh)

            def sca_all(vbc):
                # first-half score partial, batched over all 4 batches
                sca = work.tile([P, BL, S], F16, tag="sca", bufs=1)
                nc.vector.tensor_scalar_mul(sca[:], th_all[:, 0, :, :],
                                            vbc[:, 0:1])
                for j in range(1, KC):
                    nc.vector.scalar_tensor_tensor(
                        sca[:], th_all[:, j, :, :], vbc[:, j:j + 1], sca[:],
                        MULT, ADD)
                return sca

            def zout(br, b, e_t, nat_other, nat_self, out_d):
                # out = self + (E_T.T @ other) / Z, Z via ones-matmul.
                zp = ps_z.tile([P, 2], F32, tag="z", name=f"z{br}b{b}")
                nc.tensor.matmul(zp[:], e_t[:, b, :], ones_t[:],
                                 start=True, stop=True)
                rz = work.tile([P, 1], F32, tag="rz", bufs=4)
                nc.vector.reciprocal(rz[:], zp[:, 0:1])
                rp = ps_o.tile([P, H], F32, tag="o", name=f"o{br}b{b}")
                nc.tensor.matmul(rp[:], e_t[:, b, :], nat_other[:, b * H:(b + 1) * H],
                                 start=True, stop=True)
                ot = work.tile([P, H], F16, tag="ot", bufs=4)
                nc.vector.scalar_tensor_tensor(
                    ot[:], rp[:], rz[:, 0:1], nat_self[:, b * H:(b + 1) * H],
                    MULT, ADD)
                nc.sync.dma_start(out=out_d.ap()[b * P:(b + 1) * P, :],
                                  in_=ot[:])

            e_t0 = work.tile([P, BL, S], F16, tag="e0", bufs=1)
            e_t1 = work.tile([P, BL, S], F16, tag="e1", bufs=1)

            # ================= branch 0 (aspect) ========================
            for k in stage1(0, g0_head):
                if k == 5:
                    stage2_ck(0, xp_lhsT, 0, KC)
                    sca0 = sca_all(vbc_a)
            # br1 G prefetch; WAR-gated on br0-s1 reads
            nc.sync.dma_start(out=g_k0[:], in_=g_pa_d.ap()[:, 0:4 * H])
            nc.sync.dma_start(out=g_k1[:], in_=g_pa_d.ap()[:, 4 * H:8 * H])
            nc.sync.dma_start(out=xa_nat[:], in_=xa_nat_d.ap()[:])
            for i in range(6):
                load_g_k(g_pa_d, i)
            stage2_ck(0, xp_lhsT, KC, KC)
            # batched second half + exp (runs under br1-s1)
            scb0 = work.tile([P, BL, S], F16, tag="scb", bufs=1)
            nc.vector.tensor_scalar_mul(scb0[:], th_all[:, KC, :, :],
                                        vbc_a[:, KC:KC + 1])
            for j in range(1, KC):
                nc.vector.scalar_tensor_tensor(
                    scb0[:], th_all[:, KC + j, :, :],
                    vbc_a[:, KC + j:KC + j + 1], scb0[:], MULT, ADD)
            nc.vector.tensor_tensor(scb0[:], sca0[:], scb0[:], ADD)
            nc.scalar.activation(e_t0[:], scb0[:], Exp)

            # ================= branch 1 (polarity) ======================
            # br1 stage 1 with br1-s2-ck0 spliced at k=5 and br0's z/out
            # matmuls interleaved so the PE never waits on softmax chains.
            sca1b = work.tile([P, BL, S], F16, tag="sca1b", bufs=1)
            zo = 0
            for k in stage1(1, g0_tile):
                if k == 5:
                    stage2_ck(1, xa_lhsT, 0, KC)
                    sca1 = sca_all(vbc_p)
                elif k == 6:
                    # k4,k5 stage-2 spliced here so the tail chunk is k6,k7
                    stage2_ck(1, xa_lhsT, KC, 2)
                elif k == 7:
                    # fold k4,k5 into the score partial (batched, on vector)
                    nc.vector.scalar_tensor_tensor(
                        sca1b[:], th_all[:, KC, :, :], vbc_p[:, KC:KC + 1],
                        sca1[:], MULT, ADD)
                    nc.vector.scalar_tensor_tensor(
                        sca1b[:], th_all[:, KC + 1, :, :],
                        vbc_p[:, KC + 1:KC + 2], sca1b[:], MULT, ADD)
                    zout(0, zo, e_t0, xp_nat, xa_nat, out_a_d)
                    zo += 1
                elif k in (2, 3, 4):
                    zout(0, zo, e_t0, xp_nat, xa_nat, out_a_d)
                    zo += 1

            # ---- tail: k6,k7 pair-at-a-time, 2-op fp16 vector chain + exp -
            def chain_pair(pr):
                bs2 = slice(2 * pr, 2 * pr + 2)
                cha = work.tile([P, 2, S], F16, tag=f"cha{pr}", bufs=1)
                nc.vector.scalar_tensor_tensor(
                    cha[:], th_all[:, KC + 2, bs2, :], vbc_p[:, KC + 2:KC + 3],
                    sca1b[:, bs2, :], MULT, ADD)
                nc.vector.scalar_tensor_tensor(
                    cha[:], th_all[:, KC + 3, bs2, :], vbc_p[:, KC + 3:KC + 4],
                    cha[:], MULT, ADD)
                nc.scalar.activation(e_t1[:, bs2, :], cha[:], Exp)
                return cha

            Copy = mybir.ActivationFunctionType.Copy

            def ztail(b, on_vector):
                zp = ps_z.tile([P, 2], F32, tag="z", name=f"z1b{b}")
                nc.tensor.matmul(zp[:], e_t1[:, b, :], ones_t[:],
                                 start=True, stop=True)
                rz = work.tile([P, 1], F32, tag="rz", bufs=4)
                nc.vector.reciprocal(rz[:], zp[:, 0:1])
                rp = ps_o.tile([P, H], F32, tag="o", name=f"o1b{b}")
                nc.tensor.matmul(rp[:], e_t1[:, b, :], xa_nat[:, b * H:(b + 1) * H],
                                 start=True, stop=True)
                if on_vector:
                    otf = work.tile([P, H], F16, tag=f"otf{b}", bufs=1)
                    nc.vector.scalar_tensor_tensor(
                        otf[:], rp[:], rz[:, 0:1], xp_nat[:, b * H:(b + 1) * H],
                        MULT, ADD)
                else:
                    # keep vector free for the pair-1 chain: scale on the
                    # act engine, residual-add on gpsimd (slack batches)
                    ots = work.tile([P, H], F32, tag=f"ots{b}", bufs=1)
                    nc.scalar.activation(ots[:], rp[:], Copy, scale=rz[:, 0:1])
                    otf = work.tile([P, H], F16, tag=f"otf{b}", bufs=1)
                    nc.gpsimd.tensor_tensor(
                        otf[:], ots[:], xp_nat[:, b * H:(b + 1) * H], ADD)
                nc.sync.dma_start(out=out_p_d.ap()[b * P:(b + 1) * P, :],
                                  in_=otf[:])

            def filler_dep(name, lhsT, rhs):
                # PE keep-alive matmul gated on a real dependency so the
                # tile scheduler cannot hoist it ahead of the wait
                acc = ps_acc.tile([P, BL, S], F32, tag="acc", name=name)
                nc.tensor.matmul(acc[:], lhsT, rhs, start=True, stop=True)

            stage2_ck(1, xa_lhsT, KC + 2, 2, bs=(0, 1))
            cha0 = chain_pair(0)              # under b2/b3 matmuls
            stage2_ck(1, xa_lhsT, KC + 2, 2, bs=(2, 3))
            # PE keep-alives bridging the exp latencies (gated so the
            # scheduler cannot hoist them before the waits they cover)
            filler_dep("f0", th_all[:, KC + 2, 1, :],
                       th_all[:, KC + 2:KC + 4, 0:2, :])
            filler_dep("f1", cha0[:, 0, :], th_all[:, KC:2 * KC, 0, :])
            cha1 = chain_pair(1)              # vector, right after tanh b3
            ztail(0, on_vector=False)
            ztail(1, on_vector=False)
            filler_dep("f2", cha1[:, 0, :], th_all[:, KC:2 * KC, 2, :])
            filler_dep("f3", cha1[:, 1, :], th_all[:, KC:2 * KC, 3, :])
            ztail(2, on_vector=True)
            ztail(3, on_vector=True)

    nc.compile()
    return nc


def _get_nc():
    if "nc" not in _cache:
        _cache["nc"] = _build()
    return _cache["nc"]


def _prep_in_maps(aspect_hidden, polarity_hidden, G_aspect_polarity,
                  G_polarity_aspect, G_vector_aspect, G_vector_polarity):
    f16 = np.float16

    def shuffle_g(g):
        # host-side image of the SBUF G tiles, concatenated in consumption
        # order: k0 h-major block, k1 h-major block, then per-k blocks
        gr = np.asarray(g, f16).reshape(HT, P, K, H)
        pieces = [gr[:, :, k, :].transpose(1, 0, 2).reshape(P, HT * H)
                  for k in range(K)]
        return np.ascontiguousarray(np.concatenate(pieces, axis=1))

    def shuffle_t(x_loc):
        # [BS,H] -> transposed partition-major [P, (ht, bs)]
        return np.ascontiguousarray(
            x_loc.T.reshape(HT, P, BS).transpose(1, 0, 2).reshape(P, HT * BS))

    def shuffle_nat(x_loc):
        # [BS,H] -> partition-major [P, (b, h)]
        return np.ascontiguousarray(
            x_loc.reshape(BL, P, H).transpose(1, 0, 2).reshape(P, BL * H))

    a = np.asarray(aspect_hidden, f16)
    p = np.asarray(polarity_hidden, f16)
    g_ap = shuffle_g(G_aspect_polarity)
    g_pa = shuffle_g(G_polarity_aspect)
    v_ap = np.ascontiguousarray(G_vector_aspect, np.float32)
    v_pa = np.ascontiguousarray(G_vector_polarity, np.float32)

    in_maps = []
    for c in range(NCORES):
        a_loc = a[c * BL:(c + 1) * BL].reshape(BS, H)
        p_loc = p[c * BL:(c + 1) * BL].reshape(BS, H)
        xa_t = shuffle_t(a_loc)
        m = {
            "head_a": np.ascontiguousarray(
                np.concatenate([xa_t[:, 0:2 * BS], g_ap[:, 0:2 * H]], axis=1)),
            "head_b": np.ascontiguousarray(
                np.concatenate([xa_t[:, 2 * BS:], g_ap[:, 2 * H:4 * H]], axis=1)),
            "xp_t": shuffle_t(p_loc),
            "xa_nat": shuffle_nat(a_loc),
            "xp_nat": shuffle_nat(p_loc),
            "g_ap": g_ap,
            "g_pa": g_pa,
            "v_ap": v_ap,
            "v_pa": v_pa,
        }
        in_maps.append(m)
    return in_maps


def kernel(aspect_hidden, polarity_hidden, G_aspect_polarity,
           G_polarity_aspect, G_vector_aspect, G_vector_polarity):
    nc = _get_nc()
    in_maps = _prep_in_maps(aspect_hidden, polarity_hidden, G_aspect_polarity,
                            G_polarity_aspect, G_vector_aspect,
                            G_vector_polarity)
    res = run_bass_kernel_spmd(
        nc, in_maps, core_ids=list(range(NCORES)),
        trace=bool(os.environ.get("KERNEL_TRACE")))
    _cache["last_results"] = res

    out_a = np.empty((B, S, H), np.float32)
    out_p = np.empty((B, S, H), np.float32)
    for c in range(NCORES):
        out_a[c * BL:(c + 1) * BL] = res.results[c]["out_a"].astype(
            np.float32).reshape(BL, S, H)
        out_p[c * BL:(c + 1) * BL] = res.results[c]["out_p"].astype(
            np.float32).reshape(BL, S, H)
    return (out_a, out_p)


# revision 23
# speedup vs baseline: 1.1634x; 1.1634x over previous
"""CrossSharedUnit Trainium2 kernel — 8-core data-parallel over batch.

Reference computation (per batch b, S=128 tokens, H=512 hidden, K=8):
  proj[b,s,k,g] = sum_h left[b,s,h] * G[h,k,g]
  raw[b,s,t,k]  = tanh(sum_g proj[b,s,k,g] * right[b,t,g])
  score[b,s,t]  = sum_k raw[b,s,t,k] * v[k]
  attn          = softmax(score, axis=t)
  out           = self + attn @ other_hidden
for two branches (aspect: left=aspect, right=polarity; polarity: left=aspect,
right=aspect — faithful to the source which uses aspect on BOTH sides).

Sharding: batch B=32 split 4-per-core across 8 cores; G tensors replicated.
No collectives.

Precision: all matmul operands are fp16 with fp32 PSUM accumulation — fp16's
mantissa matches the fp32r (TF32) datapath the fp32 version would use, so
accuracy is unchanged (measured ~4e-3 rel err, gate 2e-2) while every input
transfer halves (fp16 also avoids fp32r's free<256 penalty and halves
LDWEIGHTS time). exp() cannot overflow fp16: |score| <= sum|v_k| ~ 7.7.
Outputs are stored fp16 and widened to fp32 on the host.

Schedule: the PE (tensor engine) is the bottleneck (~72us of matmul), so the
program is one continuous PE stream:
  warmup | br0-s1 (+ br0-s2 k0-3 spliced at k=5) | br0-s2 k4-7
         | br1-s1 (+ br1-s2 k0-3 at k=5, k4-5 at k=6, br0 z/out interleaved)
         | br1-s2 k6-7 (pair-at-a-time, z/out + dep-gated fillers between)
with softmax chains on vector/scalar underneath the next phase's matmuls.
The PE clock drops to 1.2GHz after any idle gap and takes ~5us of busy time
to recover, so warmups cover the DMA lead-in and dependency-gated filler
matmuls bridge the tail's exp latencies.

DMA: all input loads ride the sync queue in exact consumption order (one
queue, so nothing steals DMA engines from the startup-critical stream). The
k0 G block is packed interleaved with xa in two head images so the first
dma_starts cover >=4KB-contiguous partition rows (small descriptors halve
early DMA throughput), letting the first real matmul start right as the
warmup ramp ends. Stage-2's last chunk is split (k4-5 spliced into stage 1,
k6-7 at the tail) so the final softmax chain is only 2 ops deep.

The softmax division is deferred through the attention matmul:
out = self + (E @ other) / Z with Z from a ones-matmul.
"""

import os
import sys

sys.path.insert(0, "/opt/trn_rl_repo")

import numpy as np

from concourse import bacc, mybir, tile
from concourse.bass_utils import run_bass_kernel_spmd

B, S, H, K = 32, 128, 512, 8
NCORES = 8
BL = B // NCORES          # batches per core
BS = BL * S               # rows per core (512)
P = 128                   # partitions
HT = H // P               # h partition-tiles (4)
KG = K * H                # flattened (k,g) axis (4096)
KC = K // 2               # k's per stage-2 chunk (4)
F32 = mybir.dt.float32
F32R = mybir.dt.float32r
F16 = mybir.dt.float16

_cache = {}


def _build():
    """Build + compile the per-core Bass program (same program on all cores)."""
    nc = bacc.Bacc("TRN2", target_bir_lowering=False, debug=False,
                   num_devices=NCORES)

    # head images: [xa_h0|xa_h1|g_k0h0|g_k0h1] and the h2/h3 twin (fp16)
    head_a_d = nc.dram_tensor("head_a", [P, 2 * BS + 2 * H], F16,
                              kind="ExternalInput")
    head_b_d = nc.dram_tensor("head_b", [P, 2 * BS + 2 * H], F16,
                              kind="ExternalInput")
    xp_t_d = nc.dram_tensor("xp_t", [P, HT * BS], F16, kind="ExternalInput")
    xa_nat_d = nc.dram_tensor("xa_nat", [P, BL * H], F16, kind="ExternalInput")
    xp_nat_d = nc.dram_tensor("xp_nat", [P, BL * H], F16, kind="ExternalInput")
    # G: one pre-shuffled tensor per branch; column blocks in consumption
    # order: k0 h-major block (4H), k1 h-major block (4H), then per-k blocks
    g_ap_d = nc.dram_tensor("g_ap", [P, HT * KG], F16, kind="ExternalInput")
    g_pa_d = nc.dram_tensor("g_pa", [P, HT * KG], F16, kind="ExternalInput")
    v_ap_d = nc.dram_tensor("v_ap", [K, 1], F32, kind="ExternalInput")
    v_pa_d = nc.dram_tensor("v_pa", [K, 1], F32, kind="ExternalInput")
    out_a_d = nc.dram_tensor("out_a", [BS, H], F16, kind="ExternalOutput")
    out_p_d = nc.dram_tensor("out_p", [BS, H], F16, kind="ExternalOutput")

    Tanh = mybir.ActivationFunctionType.Tanh
    Exp = mybir.ActivationFunctionType.Exp
    MULT = mybir.AluOpType.mult
    ADD = mybir.AluOpType.add

    with tile.TileContext(nc) as tc:
        with (
            tc.tile_pool(name="const", bufs=1) as cpool,
            tc.tile_pool(name="g", bufs=1) as gpool,
            tc.tile_pool(name="proj", bufs=1) as projpool,
            tc.tile_pool(name="work", bufs=2) as work,
            tc.tile_pool(name="ps_acc", bufs=5, space="PSUM") as ps_acc,
            tc.tile_pool(name="ps_o", bufs=2, space="PSUM") as ps_o,
            tc.tile_pool(name="ps_z", bufs=1, space="PSUM") as ps_z,
        ):
            # ---- constants + warmup weights --------------------------------
            wm = cpool.tile([P, BS], F32R, tag="wm")
            nc.vector.memset(wm[:].bitcast(F32), 0.0)
            ones_t = cpool.tile([P, 2], F16, tag="ones_t")
            nc.vector.memset(ones_t[:], 1.0)

            # ---- persistent activations ------------------------------------
            head_a = cpool.tile([P, 2 * BS + 2 * H], F16, tag="head_a")
            head_b = cpool.tile([P, 2 * BS + 2 * H], F16, tag="head_b")
            xp_t = cpool.tile([P, HT * BS], F16, tag="xp_t")
            xa_nat = cpool.tile([P, BL * H], F16, tag="xa_nat")
            xp_nat = cpool.tile([P, BL * H], F16, tag="xp_nat")

            vrow_a = cpool.tile([1, K], F32, tag="vrow_a")
            vrow_p = cpool.tile([1, K], F32, tag="vrow_p")
            vbc_a = cpool.tile([P, K], F32, tag="vbc_a")
            vbc_p = cpool.tile([P, K], F32, tag="vbc_p")

            # ---- G piece tiles (bufs=1; k0/k1 recycled for branch 1) -------
            g_k0 = gpool.tile([P, HT * H], F16, tag="g_k0")
            g_k1 = gpool.tile([P, HT * H], F16, tag="g_k1")
            g_kk = [gpool.tile([P, HT * H], F16, tag=f"g_kk{i}",
                               name=f"g_kk{i}")
                    for i in range(6)]

            def load_g_k(g_d, i):
                o = (2 + i) * HT * H
                nc.sync.dma_start(out=g_kk[i][:], in_=g_d.ap()[:, o:o + HT * H])

            # ---- startup loads, consumption order. The first k1/k2 pieces
            # ride the gpsimd/scalar/vector queues so their issue+transfer
            # overlaps the sync queue's head stream (issues serialize ~1us
            # per dma_start within one queue).
            nc.sync.dma_start(out=head_a[:], in_=head_a_d.ap()[:])
            nc.sync.dma_start(out=head_b[:], in_=head_b_d.ap()[:])
            nc.gpsimd.dma_start(out=vrow_a[:], in_=v_ap_d.ap().rearrange("k o -> o k"))
            nc.gpsimd.dma_start(out=vrow_p[:], in_=v_pa_d.ap().rearrange("k o -> o k"))
            nc.gpsimd.partition_broadcast(vbc_a[:], vrow_a[:])
            nc.gpsimd.partition_broadcast(vbc_p[:], vrow_p[:])
            nc.sync.dma_start(out=g_k1[:], in_=g_ap_d.ap()[:, 4 * H:8 * H])
            for i in range(4):
                load_g_k(g_ap_d, i)
            nc.sync.dma_start(out=xp_t[:], in_=xp_t_d.ap()[:])
            load_g_k(g_ap_d, 4)
            load_g_k(g_ap_d, 5)
            nc.sync.dma_start(out=xp_nat[:], in_=xp_nat_d.ap()[:])
            # (g_pa loads are emitted after br0-s1 so the WAR deps pick up
            #  br0's reads; xa_nat after those.)

            def xa_rhs(h):
                t = head_a if h < 2 else head_b
                return t[:, (h % 2) * BS:(h % 2 + 1) * BS]

            def xa_lhsT(gi, b):
                t = head_a if gi < 2 else head_b
                o = (gi % 2) * BS + b * S
                return t[:, o:o + S]

            def xp_lhsT(gi, b):
                o = gi * BS + b * S
                return xp_t[:, o:o + S]

            def g0_head(h, gt):
                # branch-0 k0 weights live in the head images
                t = head_a if h < 2 else head_b
                o = 2 * BS + (h % 2) * H + gt * P
                return t[:, o:o + P]

            def g0_tile(h, gt):
                return g_k0[:, h * H + gt * P:h * H + gt * P + P]

            def g_lhsT(k, h, gt, g0_at):
                if k == 0:
                    return g0_at(h, gt)
                piece = g_k1 if k == 1 else g_kk[k - 2]
                o = h * H + gt * P
                return piece[:, o:o + P]

            # projT2[br][gt][g_part, k, b, s] — stage-1 output, stage-2 rhs;
            # per-branch buffers so br1's evacs never WAR-wait on br0-s2.
            projT2 = [[projpool.tile([P, K, BL, S], F16,
                                     tag=f"projT2_{br}_{gt}",
                                     name=f"projT2_{br}_{gt}")
                       for gt in range(HT)]
                      for br in range(2)]

            evac_state = [0]

            def evac(dst, src):
                # ping-pong PSUM evacuations between vector and scalar
                if evac_state[0] % 2 == 0:
                    nc.vector.tensor_copy(dst, src)
                else:
                    nc.scalar.copy(dst, src)
                evac_state[0] += 1

            # ---- PE warmup: get the p-state ramp going during DMA lead-in
            for w in range(7):
                acc = ps_acc.tile([P, BL, S], F32, tag="acc", name=f"warm{w}")
                nc.tensor.matmul(acc[:], wm[:, 0:P], wm[:],
                                 start=True, stop=True)

            def filler(name):
                acc = ps_acc.tile([P, BL, S], F32, tag="acc", name=name)
                nc.tensor.matmul(acc[:], wm[:, 0:P], wm[:],
                                 start=True, stop=True)

            def stage1(br, g0_at):
                # k0 h-outer with 4 open accumulators: first matmuls need
                # only the head_a image.
                accs = [ps_acc.tile([P, BL, S], F32, tag="acc",
                                    name=f"s1a{br}k0g{gt}")
                        for gt in range(HT)]
                for h in range(HT):
                    for gt in range(HT):
                        nc.tensor.matmul(
                            accs[gt][:], g_lhsT(0, h, gt, g0_at), xa_rhs(h),
                            start=(h == 0), stop=(h == HT - 1),
                            skip_group_check=True)
                for gt in range(HT):
                    evac(projT2[br][gt][:, 0, :, :], accs[gt][:])
                for k in range(1, K):
                    for gt in range(HT):
                        acc = ps_acc.tile([P, BL, S], F32, tag="acc",
                                          name=f"s1a{br}k{k}g{gt}")
                        for h in range(HT):
                            nc.tensor.matmul(
                                acc[:], g_lhsT(k, h, gt, g0_at), xa_rhs(h),
                                start=(h == 0), stop=(h == HT - 1))
                        evac(projT2[br][gt][:, k, :, :], acc[:])
                    yield k

            # th_all[t_part, k, b, s]: tanh(stage-2) output, both branches
            # (WAR-recycled). Score ops slice [:, j, :, :] batched over b.
            th_all = work.tile([P, K, BL, S], F16, tag="th", bufs=1)

            def stage2_ck(br, lhsT_of, klo, nk, bs=range(BL)):
                # raw[t, k, s] = tanh(sum_g right[t,g] proj[g,k,s]) per batch
                for b in bs:
                    acc2 = ps_acc.tile([P, nk, S], F32, tag="acc",
                                       name=f"s2a{br}b{b}c{klo}")
                    for gi in range(HT):
                        nc.tensor.matmul(
                            acc2[:],
                            lhsT_of(gi, b),
                            projT2[br][gi][:, klo:klo + nk, b, :],
                            start=(gi == 0), stop=(gi == HT - 1))
                    nc.scalar.activation(
                        th_all[:, klo:klo + nk, b, :], acc2[:], Tanh)

            def sca_all(vbc):
                # first-half score partial, batched over all 4 batches
                sca = work.tile([P, BL, S], F16, tag="sca", bufs=1)
                nc.vector.tensor_scalar_mul(sca[:], th_all[:, 0, :, :],
                                            vbc[:, 0:1])
                for j in range(1, KC):
                    nc.vector.scalar_tensor_tensor(
                        sca[:], th_all[:, j, :, :], vbc[:, j:j + 1], sca[:],
                        MULT, ADD)
                return sca

            def zout(br, b, e_t, nat_other, nat_self, out_d):
                # out = self + (E_T.T @ other) / Z, Z via ones-matmul.
                zp = ps_z.tile([P, 2], F32, tag="z", name=f"z{br}b{b}")
                nc.tensor.matmul(zp[:], e_t[:, b, :], ones_t[:],
                                 start=True, stop=True)
                rz = work.tile([P, 1], F32, tag="rz", bufs=4)
                nc.vector.reciprocal(rz[:], zp[:, 0:1])
                rp = ps_o.tile([P, H], F32, tag="o", name=f"o{br}b{b}")
                nc.tensor.matmul(rp[:], e_t[:, b, :], nat_other[:, b * H:(b + 1) * H],
                                 start=True, stop=True)
                ot = work.tile([P, H], F16, tag="ot", bufs=4)
                nc.vector.scalar_tensor_tensor(
                    ot[:], rp[:], rz[:, 0:1], nat_self[:, b * H:(b + 1) * H],
                    MULT, ADD)
                nc.sync.dma_start(out=out_d.ap()[b * P:(b + 1) * P, :],
                                  in_=ot[:])

            e_t0 = work.tile([P, BL, S], F16, tag="e0", bufs=1)
            e_t1 = work.tile([P, BL, S], F16, tag="e1", bufs=1)

            # ================= branch 0 (aspect) ========================
            for k in stage1(0, g0_head):
                if k == 5:
                    stage2_ck(0, xp_lhsT, 0, KC)
                    sca0 = sca_all(vbc_a)
            # br1 G prefetch; WAR-gated on br0-s1 reads
            nc.sync.dma_start(out=g_k0[:], in_=g_pa_d.ap()[:, 0:4 * H])
            nc.sync.dma_start(out=g_k1[:], in_=g_pa_d.ap()[:, 4 * H:8 * H])
            nc.sync.dma_start(out=xa_nat[:], in_=xa_nat_d.ap()[:])
            for i in range(6):
                load_g_k(g_pa_d, i)
            stage2_ck(0, xp_lhsT, KC, KC)
            # batched second half + exp (runs under br1-s1)
            scb0 = work.tile([P, BL, S], F16, tag="scb", bufs=1)
            nc.vector.tensor_scalar_mul(scb0[:], th_all[:, KC, :, :],
                                        vbc_a[:, KC:KC + 1])
            for j in range(1, KC):
                nc.vector.scalar_tensor_tensor(
                    scb0[:], th_all[:, KC + j, :, :],
                    vbc_a[:, KC + j:KC + j + 1], scb0[:], MULT, ADD)
            nc.vector.tensor_tensor(scb0[:], sca0[:], scb0[:], ADD)
            nc.scalar.activation(e_t0[:], scb0[:], Exp)

            # ================= branch 1 (polarity) ======================
            # br1 stage 1 with br1-s2-ck0 spliced at k=5 and br0's z/out
            # matmuls interleaved so the PE never waits on softmax chains.
            sca1b = work.tile([P, BL, S], F16, tag="sca1b", bufs=1)
            zo = 0
            for k in stage1(1, g0_tile):
                if k == 5:
                    stage2_ck(1, xa_lhsT, 0, KC)
                    sca1 = sca_all(vbc_p)
                elif k == 6:
                    # k4,k5 stage-2 spliced here so the tail chunk is k6,k7
                    stage2_ck(1, xa_lhsT, KC, 2)
                elif k == 7:
                    # fold k4,k5 into the score partial (batched, on vector)
                    nc.vector.scalar_tensor_tensor(
                        sca1b[:], th_all[:, KC, :, :], vbc_p[:, KC:KC + 1],
                        sca1[:], MULT, ADD)
                    nc.vector.scalar_tensor_tensor(
                        sca1b[:], th_all[:, KC + 1, :, :],
                        vbc_p[:, KC + 1:KC + 2], sca1b[:], MULT, ADD)
                    zout(0, zo, e_t0, xp_nat, xa_nat, out_a_d)
                    zo += 1
                elif k in (2, 3, 4):
                    zout(0, zo, e_t0, xp_nat, xa_nat, out_a_d)
                    zo += 1

            # ---- tail: k6,k7 pair-at-a-time, 2-op fp16 vector chain + exp -
            def chain_pair(pr):
                bs2 = slice(2 * pr, 2 * pr + 2)
                cha = work.tile([P, 2, S], F16, tag=f"cha{pr}", bufs=1)
                nc.vector.scalar_tensor_tensor(
                    cha[:], th_all[:, KC + 2, bs2, :], vbc_p[:, KC + 2:KC + 3],
                    sca1b[:, bs2, :], MULT, ADD)
                nc.vector.scalar_tensor_tensor(
                    cha[:], th_all[:, KC + 3, bs2, :], vbc_p[:, KC + 3:KC + 4],
                    cha[:], MULT, ADD)
                nc.scalar.activation(e_t1[:, bs2, :], cha[:], Exp)
                return cha

            Copy = mybir.ActivationFunctionType.Copy

            def ztail(b, on_vector):
                zp = ps_z.tile([P, 2], F32, tag="z", name=f"z1b{b}")
                nc.tensor.matmul(zp[:], e_t1[:, b, :], ones_t[:],
                                 start=True, stop=True)
                rz = work.tile([P, 1], F32, tag="rz", bufs=4)
                nc.vector.reciprocal(rz[:], zp[:, 0:1])
                rp = ps_o.tile([P, H], F32, tag="o", name=f"o1b{b}")
                nc.tensor.matmul(rp[:], e_t1[:, b, :], xa_nat[:, b * H:(b + 1) * H],
                                 start=True, stop=True)
                if on_vector:
                    otf = work.tile([P, H], F16, tag=f"otf{b}", bufs=1)
                    nc.vector.scalar_tensor_tensor(
                        otf[:], rp[:], rz[:, 0:1], xp_nat[:, b * H:(b + 1) * H],
                        MULT, ADD)
                else:
                    # keep vector free for the pair-1 chain: scale on the
                    # act engine, residual-add on gpsimd (slack batches)
                    ots = work.tile([P, H], F32, tag=f"ots{b}", bufs=1)
                    nc.scalar.activation(ots[:], rp[:], Copy, scale=rz[:, 0:1])
                    otf = work.tile([P, H], F16, tag=f"otf{b}", bufs=1)
                    nc.gpsimd.tensor_tensor(
                        otf[:], ots[:], xp_nat[:, b * H:(b + 1) * H], ADD)
                nc.sync.dma_start(out=out_p_d.ap()[b * P:(b + 1) * P, :],
                                  in_=otf[:])

            def filler_dep(name, lhsT, rhs):
                # PE keep-alive matmul gated on a real dependency so the
                # tile scheduler cannot hoist it ahead of the wait
                acc = ps_acc.tile([P, BL, S], F32, tag="acc", name=name)
                nc.tensor.matmul(acc[:], lhsT, rhs, start=True, stop=True)

            stage2_ck(1, xa_lhsT, KC + 2, 2, bs=(0, 1))
            cha0 = chain_pair(0)              # under b2/b3 matmuls
            stage2_ck(1, xa_lhsT, KC + 2, 2, bs=(2, 3))
            # PE keep-alives bridging the exp latencies (gated so the
            # scheduler cannot hoist them before the waits they cover)
            filler_dep("f0", th_all[:, KC + 2, 1, :],
                       th_all[:, KC + 2:KC + 4, 0:2, :])
            filler_dep("f1", cha0[:, 0, :], th_all[:, KC:2 * KC, 0, :])
            cha1 = chain_pair(1)              # vector, right after tanh b3
            ztail(0, on_vector=False)
            ztail(1, on_vector=False)
            filler_dep("f2", cha1[:, 0, :], th_all[:, KC:2 * KC, 2, :])
            filler_dep("f3", cha1[:, 1, :], th_all[:, KC:2 * KC, 3, :])
            ztail(2, on_vector=True)
            ztail(3, on_vector=True)

    nc.compile()
    return nc


def _get_nc():
    if "nc" not in _cache:
        _cache["nc"] = _build()
    return _cache["nc"]


def _prep_in_maps(aspect_hidden, polarity_hidden, G_aspect_polarity,
                  G_polarity_aspect, G_vector_aspect, G_vector_polarity):
    f16 = np.float16

    def shuffle_g(g):
        # host-side image of the SBUF G tiles, concatenated in consumption
        # order: k0 h-major block, k1 h-major block, then per-k blocks
        gr = np.asarray(g, f16).reshape(HT, P, K, H)
        pieces = [gr[:, :, k, :].transpose(1, 0, 2).reshape(P, HT * H)
                  for k in range(K)]
        return np.ascontiguousarray(np.concatenate(pieces, axis=1))

    def shuffle_t(x_loc):
        # [BS,H] -> transposed partition-major [P, (ht, bs)]
        return np.ascontiguousarray(
            x_loc.T.reshape(HT, P, BS).transpose(1, 0, 2).reshape(P, HT * BS))

    def shuffle_nat(x_loc):
        # [BS,H] -> partition-major [P, (b, h)]
        return np.ascontiguousarray(
            x_loc.reshape(BL, P, H).transpose(1, 0, 2).reshape(P, BL * H))

    a = np.asarray(aspect_hidden, f16)
    p = np.asarray(polarity_hidden, f16)
    g_ap = shuffle_g(G_aspect_polarity)
    g_pa = shuffle_g(G_polarity_aspect)
    v_ap = np.ascontiguousarray(G_vector_aspect, np.float32)
    v_pa = np.ascontiguousarray(G_vector_polarity, np.float32)

    in_maps = []
    for c in range(NCORES):
        a_loc = a[c * BL:(c + 1) * BL].reshape(BS, H)
        p_loc = p[c * BL:(c + 1) * BL].reshape(BS, H)
        xa_t = shuffle_t(a_loc)
        m = {
            "head_a": np.ascontiguousarray(
                np.concatenate([xa_t[:, 0:2 * BS], g_ap[:, 0:2 * H]], axis=1)),
            "head_b": np.ascontiguousarray(
                np.concatenate([xa_t[:, 2 * BS:], g_ap[:, 2 * H:4 * H]], axis=1)),
            "xp_t": shuffle_t(p_loc),
            "xa_nat": shuffle_nat(a_loc),
            "xp_nat": shuffle_nat(p_loc),
            "g_ap": g_ap,
            "g_pa": g_pa,
            "v_ap": v_ap,
            "v_pa": v_pa,
        }
        in_maps.append(m)
    return in_maps


def kernel(aspect_hidden, polarity_hidden, G_aspect_polarity,
           G_polarity_aspect, G_vector_aspect, G_vector_polarity):
    nc = _get_nc()
    in_maps = _prep_in_maps(aspect_hidden, polarity_hidden, G_aspect_polarity,
                            G_polarity_aspect, G_vector_aspect,
                            G_vector_polarity)
    res = run_bass_kernel_spmd(
        nc, in_maps, core_ids=list(range(NCORES)),
        trace=bool(os.environ.get("KERNEL_TRACE")))
    _cache["last_results"] = res

    out_a = np.empty((B, S, H), np.float32)
    out_p = np.empty((B, S, H), np.float32)
    for c in range(NCORES):
        out_a[c * BL:(c + 1) * BL] = res.results[c]["out_a"].astype(
            np.float32).reshape(BL, S, H)
        out_p[c * BL:(c + 1) * BL] = res.results[c]["out_p"].astype(
            np.float32).reshape(BL, S, H)
    return (out_a, out_p)


# revision 24
# speedup vs baseline: 1.1709x; 1.0064x over previous
"""CrossSharedUnit Trainium2 kernel — 8-core data-parallel over batch.

Reference computation (per batch b, S=128 tokens, H=512 hidden, K=8):
  proj[b,s,k,g] = sum_h left[b,s,h] * G[h,k,g]
  raw[b,s,t,k]  = tanh(sum_g proj[b,s,k,g] * right[b,t,g])
  score[b,s,t]  = sum_k raw[b,s,t,k] * v[k]
  attn          = softmax(score, axis=t)
  out           = self + attn @ other_hidden
for two branches (aspect: left=aspect, right=polarity; polarity: left=aspect,
right=aspect — faithful to the source which uses aspect on BOTH sides).

Sharding: batch B=32 split 4-per-core across 8 cores; G tensors replicated.
No collectives.

Precision: all matmul operands are fp16 with fp32 PSUM accumulation — fp16's
mantissa matches the fp32r (TF32) datapath the fp32 version would use, so
accuracy is unchanged (measured ~4e-3 rel err, gate 2e-2) while every input
transfer halves (fp16 also avoids fp32r's free<256 penalty and halves
LDWEIGHTS time). exp() cannot overflow fp16: |score| <= sum|v_k| ~ 7.7.
Outputs are stored fp16 and widened to fp32 on the host.

Schedule: the PE (tensor engine) is the bottleneck (~72us of matmul), so the
program is one continuous PE stream:
  warmup | br0-s1 (+ br0-s2 k0-3 spliced at k=5) | br0-s2 k4-7
         | br1-s1 (+ br1-s2 k0-3 at k=5, k4-5 at k=6, br0 z/out interleaved)
         | br1-s2 k6-7 (pair-at-a-time, z/out + dep-gated fillers between)
with softmax chains on vector/scalar underneath the next phase's matmuls.
The PE clock drops to 1.2GHz after any idle gap and takes ~5us of busy time
to recover, so warmups cover the DMA lead-in and dependency-gated filler
matmuls bridge the tail's exp latencies.

DMA: all input loads ride the sync queue in exact consumption order (one
queue, so nothing steals DMA engines from the startup-critical stream). The
k0 G block is packed interleaved with xa in two head images so the first
dma_starts cover >=4KB-contiguous partition rows (small descriptors halve
early DMA throughput), letting the first real matmul start right as the
warmup ramp ends. Stage-2's last chunk is split (k4-5 spliced into stage 1,
k6-7 at the tail) so the final softmax chain is only 2 ops deep.

The softmax division is deferred through the attention matmul:
out = self + (E @ other) / Z with Z from a ones-matmul.
"""

import os
import sys

sys.path.insert(0, "/opt/trn_rl_repo")

import numpy as np

from concourse import bacc, mybir, tile
from concourse.bass_utils import run_bass_kernel_spmd

B, S, H, K = 32, 128, 512, 8
NCORES = 8
BL = B // NCORES          # batches per core
BS = BL * S               # rows per core (512)
P = 128                   # partitions
HT = H // P               # h partition-tiles (4)
KG = K * H                # flattened (k,g) axis (4096)
KC = K // 2               # k's per stage-2 chunk (4)
F32 = mybir.dt.float32
F32R = mybir.dt.float32r
F16 = mybir.dt.float16

_cache = {}


def _build():
    """Build + compile the per-core Bass program (same program on all cores)."""
    nc = bacc.Bacc("TRN2", target_bir_lowering=False, debug=False,
                   num_devices=NCORES)

    # head images: [xa_h0|xa_h1|g_k0h0|g_k0h1] and the h2/h3 twin (fp16)
    head_a_d = nc.dram_tensor("head_a", [P, 2 * BS + 2 * H], F16,
                              kind="ExternalInput")
    head_b_d = nc.dram_tensor("head_b", [P, 2 * BS + 2 * H], F16,
                              kind="ExternalInput")
    xp_t_d = nc.dram_tensor("xp_t", [P, HT * BS], F16, kind="ExternalInput")
    xa_nat_d = nc.dram_tensor("xa_nat", [P, BL * H], F16, kind="ExternalInput")
    xp_nat_d = nc.dram_tensor("xp_nat", [P, BL * H], F16, kind="ExternalInput")
    # G: one pre-shuffled tensor per branch; column blocks in consumption
    # order: k0 h-major block (4H), k1 h-major block (4H), then per-k blocks
    g_ap_d = nc.dram_tensor("g_ap", [P, HT * KG], F16, kind="ExternalInput")
    g_pa_d = nc.dram_tensor("g_pa", [P, HT * KG], F16, kind="ExternalInput")
    v_ap_d = nc.dram_tensor("v_ap", [K, 1], F32, kind="ExternalInput")
    v_pa_d = nc.dram_tensor("v_pa", [K, 1], F32, kind="ExternalInput")
    out_a_d = nc.dram_tensor("out_a", [BS, H], F16, kind="ExternalOutput")
    out_p_d = nc.dram_tensor("out_p", [BS, H], F16, kind="ExternalOutput")

    Tanh = mybir.ActivationFunctionType.Tanh
    Exp = mybir.ActivationFunctionType.Exp
    MULT = mybir.AluOpType.mult
    ADD = mybir.AluOpType.add

    with tile.TileContext(nc) as tc:
        with (
            tc.tile_pool(name="const", bufs=1) as cpool,
            tc.tile_pool(name="g", bufs=1) as gpool,
            tc.tile_pool(name="proj", bufs=1) as projpool,
            tc.tile_pool(name="work", bufs=2) as work,
            tc.tile_pool(name="ps_acc", bufs=5, space="PSUM") as ps_acc,
            tc.tile_pool(name="ps_o", bufs=2, space="PSUM") as ps_o,
            tc.tile_pool(name="ps_z", bufs=1, space="PSUM") as ps_z,
        ):
            # ---- constants + warmup weights --------------------------------
            wm = cpool.tile([P, BS], F32R, tag="wm")
            nc.vector.memset(wm[:].bitcast(F32), 0.0)
            ones_t = cpool.tile([P, 2], F16, tag="ones_t")
            nc.vector.memset(ones_t[:], 1.0)

            # ---- persistent activations ------------------------------------
            head_a = cpool.tile([P, 2 * BS + 2 * H], F16, tag="head_a")
            head_b = cpool.tile([P, 2 * BS + 2 * H], F16, tag="head_b")
            xp_t = cpool.tile([P, HT * BS], F16, tag="xp_t")
            xa_nat = cpool.tile([P, BL * H], F16, tag="xa_nat")
            xp_nat = cpool.tile([P, BL * H], F16, tag="xp_nat")

            vrow_a = cpool.tile([1, K], F32, tag="vrow_a")
            vrow_p = cpool.tile([1, K], F32, tag="vrow_p")
            vbc_a = cpool.tile([P, K], F32, tag="vbc_a")
            vbc_p = cpool.tile([P, K], F32, tag="vbc_p")

            # ---- G piece tiles (bufs=1; k0/k1 recycled for branch 1) -------
            g_k0 = gpool.tile([P, HT * H], F16, tag="g_k0")
            g_k1 = gpool.tile([P, HT * H], F16, tag="g_k1")
            g_kk = [gpool.tile([P, HT * H], F16, tag=f"g_kk{i}",
                               name=f"g_kk{i}")
                    for i in range(6)]

            def load_g_k(g_d, i):
                o = (2 + i) * HT * H
                nc.sync.dma_start(out=g_kk[i][:], in_=g_d.ap()[:, o:o + HT * H])

            # ---- startup loads, consumption order. The first k1/k2 pieces
            # ride the gpsimd/scalar/vector queues so their issue+transfer
            # overlaps the sync queue's head stream (issues serialize ~1us
            # per dma_start within one queue).
            nc.sync.dma_start(out=head_a[:], in_=head_a_d.ap()[:])
            nc.sync.dma_start(out=head_b[:], in_=head_b_d.ap()[:])
            nc.gpsimd.dma_start(out=vrow_a[:], in_=v_ap_d.ap().rearrange("k o -> o k"))
            nc.gpsimd.dma_start(out=vrow_p[:], in_=v_pa_d.ap().rearrange("k o -> o k"))
            nc.gpsimd.partition_broadcast(vbc_a[:], vrow_a[:])
            nc.gpsimd.partition_broadcast(vbc_p[:], vrow_p[:])
            nc.sync.dma_start(out=g_k1[:], in_=g_ap_d.ap()[:, 4 * H:8 * H])
            for i in range(4):
                load_g_k(g_ap_d, i)
            nc.sync.dma_start(out=xp_t[:], in_=xp_t_d.ap()[:])
            load_g_k(g_ap_d, 4)
            load_g_k(g_ap_d, 5)
            nc.sync.dma_start(out=xp_nat[:], in_=xp_nat_d.ap()[:])
            # (g_pa loads are emitted after br0-s1 so the WAR deps pick up
            #  br0's reads; xa_nat after those.)

            def xa_rhs(h):
                t = head_a if h < 2 else head_b
                return t[:, (h % 2) * BS:(h % 2 + 1) * BS]

            def xa_lhsT(gi, b):
                t = head_a if gi < 2 else head_b
                o = (gi % 2) * BS + b * S
                return t[:, o:o + S]

            def xp_lhsT(gi, b):
                o = gi * BS + b * S
                return xp_t[:, o:o + S]

            def g0_head(h, gt):
                # branch-0 k0 weights live in the head images
                t = head_a if h < 2 else head_b
                o = 2 * BS + (h % 2) * H + gt * P
                return t[:, o:o + P]

            def g0_tile(h, gt):
                return g_k0[:, h * H + gt * P:h * H + gt * P + P]

            def g_lhsT(k, h, gt, g0_at):
                if k == 0:
                    return g0_at(h, gt)
                piece = g_k1 if k == 1 else g_kk[k - 2]
                o = h * H + gt * P
                return piece[:, o:o + P]

            # projT2[br][gt][g_part, k, b, s] — stage-1 output, stage-2 rhs;
            # per-branch buffers so br1's evacs never WAR-wait on br0-s2.
            projT2 = [[projpool.tile([P, K, BL, S], F16,
                                     tag=f"projT2_{br}_{gt}",
                                     name=f"projT2_{br}_{gt}")
                       for gt in range(HT)]
                      for br in range(2)]

            evac_state = [0]

            def evac(dst, src):
                # ping-pong PSUM evacuations between vector and scalar
                if evac_state[0] % 2 == 0:
                    nc.vector.tensor_copy(dst, src)
                else:
                    nc.scalar.copy(dst, src)
                evac_state[0] += 1

            # ---- PE warmup: get the p-state ramp going during DMA lead-in
            for w in range(5):
                acc = ps_acc.tile([P, BL, S], F32, tag="acc", name=f"warm{w}")
                nc.tensor.matmul(acc[:], wm[:, 0:P], wm[:],
                                 start=True, stop=True)

            def filler(name):
                acc = ps_acc.tile([P, BL, S], F32, tag="acc", name=name)
                nc.tensor.matmul(acc[:], wm[:, 0:P], wm[:],
                                 start=True, stop=True)

            def stage1(br, g0_at):
                # k0 h-outer with 4 open accumulators: first matmuls need
                # only the head_a image.
                accs = [ps_acc.tile([P, BL, S], F32, tag="acc",
                                    name=f"s1a{br}k0g{gt}")
                        for gt in range(HT)]
                for h in range(HT):
                    for gt in range(HT):
                        nc.tensor.matmul(
                            accs[gt][:], g_lhsT(0, h, gt, g0_at), xa_rhs(h),
                            start=(h == 0), stop=(h == HT - 1),
                            skip_group_check=True)
                for gt in range(HT):
                    evac(projT2[br][gt][:, 0, :, :], accs[gt][:])
                for k in range(1, K):
                    for gt in range(HT):
                        acc = ps_acc.tile([P, BL, S], F32, tag="acc",
                                          name=f"s1a{br}k{k}g{gt}")
                        for h in range(HT):
                            nc.tensor.matmul(
                                acc[:], g_lhsT(k, h, gt, g0_at), xa_rhs(h),
                                start=(h == 0), stop=(h == HT - 1))
                        evac(projT2[br][gt][:, k, :, :], acc[:])
                    yield k

            # th_all[t_part, k, b, s]: tanh(stage-2) output, both branches
            # (WAR-recycled). Score ops slice [:, j, :, :] batched over b.
            th_all = work.tile([P, K, BL, S], F16, tag="th", bufs=1)

            def stage2_ck(br, lhsT_of, klo, nk, bs=range(BL)):
                # raw[t, k, s] = tanh(sum_g right[t,g] proj[g,k,s]) per batch
                for b in bs:
                    acc2 = ps_acc.tile([P, nk, S], F32, tag="acc",
                                       name=f"s2a{br}b{b}c{klo}")
                    for gi in range(HT):
                        nc.tensor.matmul(
                            acc2[:],
                            lhsT_of(gi, b),
                            projT2[br][gi][:, klo:klo + nk, b, :],
                            start=(gi == 0), stop=(gi == HT - 1))
                    nc.scalar.activation(
                        th_all[:, klo:klo + nk, b, :], acc2[:], Tanh)

            def sca_all(vbc):
                # first-half score partial, batched over all 4 batches
                sca = work.tile([P, BL, S], F16, tag="sca", bufs=1)
                nc.vector.tensor_scalar_mul(sca[:], th_all[:, 0, :, :],
                                            vbc[:, 0:1])
                for j in range(1, KC):
                    nc.vector.scalar_tensor_tensor(
                        sca[:], th_all[:, j, :, :], vbc[:, j:j + 1], sca[:],
                        MULT, ADD)
                return sca

            def zout(br, b, e_t, nat_other, nat_self, out_d):
                # out = self + (E_T.T @ other) / Z, Z via ones-matmul.
                zp = ps_z.tile([P, 2], F32, tag="z", name=f"z{br}b{b}")
                nc.tensor.matmul(zp[:], e_t[:, b, :], ones_t[:],
                                 start=True, stop=True)
                rz = work.tile([P, 1], F32, tag="rz", bufs=4)
                nc.vector.reciprocal(rz[:], zp[:, 0:1])
                rp = ps_o.tile([P, H], F32, tag="o", name=f"o{br}b{b}")
                nc.tensor.matmul(rp[:], e_t[:, b, :], nat_other[:, b * H:(b + 1) * H],
                                 start=True, stop=True)
                ot = work.tile([P, H], F16, tag="ot", bufs=4)
                nc.vector.scalar_tensor_tensor(
                    ot[:], rp[:], rz[:, 0:1], nat_self[:, b * H:(b + 1) * H],
                    MULT, ADD)
                nc.sync.dma_start(out=out_d.ap()[b * P:(b + 1) * P, :],
                                  in_=ot[:])

            e_t0 = work.tile([P, BL, S], F16, tag="e0", bufs=1)
            e_t1 = work.tile([P, BL, S], F16, tag="e1", bufs=1)

            # ================= branch 0 (aspect) ========================
            for k in stage1(0, g0_head):
                if k == 5:
                    stage2_ck(0, xp_lhsT, 0, KC)
                    sca0 = sca_all(vbc_a)
            # br1 G prefetch; WAR-gated on br0-s1 reads
            nc.sync.dma_start(out=g_k0[:], in_=g_pa_d.ap()[:, 0:4 * H])
            nc.sync.dma_start(out=g_k1[:], in_=g_pa_d.ap()[:, 4 * H:8 * H])
            nc.sync.dma_start(out=xa_nat[:], in_=xa_nat_d.ap()[:])
            for i in range(6):
                load_g_k(g_pa_d, i)
            stage2_ck(0, xp_lhsT, KC, KC)
            # batched second half + exp (runs under br1-s1)
            scb0 = work.tile([P, BL, S], F16, tag="scb", bufs=1)
            nc.vector.tensor_scalar_mul(scb0[:], th_all[:, KC, :, :],
                                        vbc_a[:, KC:KC + 1])
            for j in range(1, KC):
                nc.vector.scalar_tensor_tensor(
                    scb0[:], th_all[:, KC + j, :, :],
                    vbc_a[:, KC + j:KC + j + 1], scb0[:], MULT, ADD)
            nc.vector.tensor_tensor(scb0[:], sca0[:], scb0[:], ADD)
            nc.scalar.activation(e_t0[:], scb0[:], Exp)

            # ================= branch 1 (polarity) ======================
            # br1 stage 1 with br1-s2-ck0 spliced at k=5 and br0's z/out
            # matmuls interleaved so the PE never waits on softmax chains.
            sca1b = work.tile([P, BL, S], F16, tag="sca1b", bufs=1)
            zo = 0
            for k in stage1(1, g0_tile):
                if k == 5:
                    stage2_ck(1, xa_lhsT, 0, KC)
                    sca1 = sca_all(vbc_p)
                elif k == 6:
                    # k4,k5 stage-2 spliced here so the tail chunk is k6,k7
                    stage2_ck(1, xa_lhsT, KC, 2)
                elif k == 7:
                    # fold k4,k5 into the score partial (batched, on vector)
                    nc.vector.scalar_tensor_tensor(
                        sca1b[:], th_all[:, KC, :, :], vbc_p[:, KC:KC + 1],
                        sca1[:], MULT, ADD)
                    nc.vector.scalar_tensor_tensor(
                        sca1b[:], th_all[:, KC + 1, :, :],
                        vbc_p[:, KC + 1:KC + 2], sca1b[:], MULT, ADD)
                    zout(0, zo, e_t0, xp_nat, xa_nat, out_a_d)
                    zo += 1
                elif k in (2, 3, 4):
                    zout(0, zo, e_t0, xp_nat, xa_nat, out_a_d)
                    zo += 1

            # ---- tail: k6,k7 pair-at-a-time, 2-op fp16 vector chain + exp -
            def chain_pair(pr):
                bs2 = slice(2 * pr, 2 * pr + 2)
                cha = work.tile([P, 2, S], F16, tag=f"cha{pr}", bufs=1)
                nc.vector.scalar_tensor_tensor(
                    cha[:], th_all[:, KC + 2, bs2, :], vbc_p[:, KC + 2:KC + 3],
                    sca1b[:, bs2, :], MULT, ADD)
                nc.vector.scalar_tensor_tensor(
                    cha[:], th_all[:, KC + 3, bs2, :], vbc_p[:, KC + 3:KC + 4],
                    cha[:], MULT, ADD)
                nc.scalar.activation(e_t1[:, bs2, :], cha[:], Exp)
                return cha

            Copy = mybir.ActivationFunctionType.Copy

            def ztail(b, on_vector):
                zp = ps_z.tile([P, 2], F32, tag="z", name=f"z1b{b}")
                nc.tensor.matmul(zp[:], e_t1[:, b, :], ones_t[:],
                                 start=True, stop=True)
                rz = work.tile([P, 1], F32, tag="rz", bufs=4)
                nc.vector.reciprocal(rz[:], zp[:, 0:1])
                rp = ps_o.tile([P, H], F32, tag="o", name=f"o1b{b}")
                nc.tensor.matmul(rp[:], e_t1[:, b, :], xa_nat[:, b * H:(b + 1) * H],
                                 start=True, stop=True)
                if on_vector:
                    otf = work.tile([P, H], F16, tag=f"otf{b}", bufs=1)
                    nc.vector.scalar_tensor_tensor(
                        otf[:], rp[:], rz[:, 0:1], xp_nat[:, b * H:(b + 1) * H],
                        MULT, ADD)
                else:
                    # keep vector free for the pair-1 chain: scale on the
                    # act engine, residual-add on gpsimd (slack batches)
                    ots = work.tile([P, H], F32, tag=f"ots{b}", bufs=1)
                    nc.scalar.activation(ots[:], rp[:], Copy, scale=rz[:, 0:1])
                    otf = work.tile([P, H], F16, tag=f"otf{b}", bufs=1)
                    nc.gpsimd.tensor_tensor(
                        otf[:], ots[:], xp_nat[:, b * H:(b + 1) * H], ADD)
                nc.sync.dma_start(out=out_p_d.ap()[b * P:(b + 1) * P, :],
                                  in_=otf[:])

            def filler_dep(name, lhsT, rhs):
                # PE keep-alive matmul gated on a real dependency so the
                # tile scheduler cannot hoist it ahead of the wait
                acc = ps_acc.tile([P, BL, S], F32, tag="acc", name=name)
                nc.tensor.matmul(acc[:], lhsT, rhs, start=True, stop=True)

            stage2_ck(1, xa_lhsT, KC + 2, 2, bs=(0, 1))
            cha0 = chain_pair(0)              # under b2/b3 matmuls
            stage2_ck(1, xa_lhsT, KC + 2, 2, bs=(2, 3))
            # PE keep-alives bridging the exp latencies (gated so the
            # scheduler cannot hoist them before the waits they cover)
            filler_dep("f0", th_all[:, KC + 2, 1, :],
                       th_all[:, KC + 2:KC + 4, 0:2, :])
            filler_dep("f1", cha0[:, 0, :], th_all[:, KC:2 * KC, 0, :])
            cha1 = chain_pair(1)              # vector, right after tanh b3
            ztail(0, on_vector=False)
            ztail(1, on_vector=False)
            filler_dep("f2", cha1[:, 0, :], th_all[:, KC:2 * KC, 2, :])
            filler_dep("f3", cha1[:, 1, :], th_all[:, KC:2 * KC, 3, :])
            ztail(2, on_vector=True)
            ztail(3, on_vector=True)

    nc.compile()
    return nc


def _get_nc():
    if "nc" not in _cache:
        _cache["nc"] = _build()
    return _cache["nc"]


def _prep_in_maps(aspect_hidden, polarity_hidden, G_aspect_polarity,
                  G_polarity_aspect, G_vector_aspect, G_vector_polarity):
    f16 = np.float16

    def shuffle_g(g):
        # host-side image of the SBUF G tiles, concatenated in consumption
        # order: k0 h-major block, k1 h-major block, then per-k blocks
        gr = np.asarray(g, f16).reshape(HT, P, K, H)
        pieces = [gr[:, :, k, :].transpose(1, 0, 2).reshape(P, HT * H)
                  for k in range(K)]
        return np.ascontiguousarray(np.concatenate(pieces, axis=1))

    def shuffle_t(x_loc):
        # [BS,H] -> transposed partition-major [P, (ht, bs)]
        return np.ascontiguousarray(
            x_loc.T.reshape(HT, P, BS).transpose(1, 0, 2).reshape(P, HT * BS))

    def shuffle_nat(x_loc):
        # [BS,H] -> partition-major [P, (b, h)]
        return np.ascontiguousarray(
            x_loc.reshape(BL, P, H).transpose(1, 0, 2).reshape(P, BL * H))

    a = np.asarray(aspect_hidden, f16)
    p = np.asarray(polarity_hidden, f16)
    g_ap = shuffle_g(G_aspect_polarity)
    g_pa = shuffle_g(G_polarity_aspect)
    v_ap = np.ascontiguousarray(G_vector_aspect, np.float32)
    v_pa = np.ascontiguousarray(G_vector_polarity, np.float32)

    in_maps = []
    for c in range(NCORES):
        a_loc = a[c * BL:(c + 1) * BL].reshape(BS, H)
        p_loc = p[c * BL:(c + 1) * BL].reshape(BS, H)
        xa_t = shuffle_t(a_loc)
        m = {
            "head_a": np.ascontiguousarray(
                np.concatenate([xa_t[:, 0:2 * BS], g_ap[:, 0:2 * H]], axis=1)),
            "head_b": np.ascontiguousarray(
                np.concatenate([xa_t[:, 2 * BS:], g_ap[:, 2 * H:4 * H]], axis=1)),
            "xp_t": shuffle_t(p_loc),
            "xa_nat": shuffle_nat(a_loc),
            "xp_nat": shuffle_nat(p_loc),
            "g_ap": g_ap,
            "g_pa": g_pa,
            "v_ap": v_ap,
            "v_pa": v_pa,
        }
        in_maps.append(m)
    return in_maps


def kernel(aspect_hidden, polarity_hidden, G_aspect_polarity,
           G_polarity_aspect, G_vector_aspect, G_vector_polarity):
    nc = _get_nc()
    in_maps = _prep_in_maps(aspect_hidden, polarity_hidden, G_aspect_polarity,
                            G_polarity_aspect, G_vector_aspect,
                            G_vector_polarity)
    res = run_bass_kernel_spmd(
        nc, in_maps, core_ids=list(range(NCORES)),
        trace=bool(os.environ.get("KERNEL_TRACE")))
    _cache["last_results"] = res

    out_a = np.empty((B, S, H), np.float32)
    out_p = np.empty((B, S, H), np.float32)
    for c in range(NCORES):
        out_a[c * BL:(c + 1) * BL] = res.results[c]["out_a"].astype(
            np.float32).reshape(BL, S, H)
        out_p[c * BL:(c + 1) * BL] = res.results[c]["out_p"].astype(
            np.float32).reshape(BL, S, H)
    return (out_a, out_p)


# revision 25
# speedup vs baseline: 1.1846x; 1.0117x over previous
"""CrossSharedUnit Trainium2 kernel — 8-core data-parallel over batch.

Reference computation (per batch b, S=128 tokens, H=512 hidden, K=8):
  proj[b,s,k,g] = sum_h left[b,s,h] * G[h,k,g]
  raw[b,s,t,k]  = tanh(sum_g proj[b,s,k,g] * right[b,t,g])
  score[b,s,t]  = sum_k raw[b,s,t,k] * v[k]
  attn          = softmax(score, axis=t)
  out           = self + attn @ other_hidden
for two branches (aspect: left=aspect, right=polarity; polarity: left=aspect,
right=aspect — faithful to the source which uses aspect on BOTH sides).

Sharding: batch B=32 split 4-per-core across 8 cores; G tensors replicated.
No collectives.

Precision: all matmul operands are fp16 with fp32 PSUM accumulation — fp16's
mantissa matches the fp32r (TF32) datapath the fp32 version would use, so
accuracy is unchanged (measured ~4e-3 rel err, gate 2e-2) while every input
transfer halves (fp16 also avoids fp32r's free<256 penalty and halves
LDWEIGHTS time). exp() cannot overflow fp16: |score| <= sum|v_k| ~ 7.7.
Outputs are stored fp16 and widened to fp32 on the host.

Schedule: the PE (tensor engine) is the bottleneck (~72us of matmul), so the
program is one continuous PE stream:
  warmup | br0-s1 (+ br0-s2 k0-3 spliced at k=5) | br0-s2 k4-7
         | br1-s1 (+ br1-s2 k0-3 at k=5, k4-5 at k=6, br0 z/out interleaved)
         | br1-s2 k6-7 (pair-at-a-time, z/out + dep-gated fillers between)
with softmax chains on vector/scalar underneath the next phase's matmuls.
The PE clock drops to 1.2GHz after any idle gap and takes ~5us of busy time
to recover, so warmups cover the DMA lead-in and dependency-gated filler
matmuls bridge the tail's exp latencies.

DMA: all input loads ride the sync queue in exact consumption order (one
queue, so nothing steals DMA engines from the startup-critical stream). The
k0 G block is packed interleaved with xa in two head images so the first
dma_starts cover >=4KB-contiguous partition rows (small descriptors halve
early DMA throughput), letting the first real matmul start right as the
warmup ramp ends. Stage-2's last chunk is split (k4-5 spliced into stage 1,
k6-7 at the tail) so the final softmax chain is only 2 ops deep.

The softmax division is deferred through the attention matmul:
out = self + (E @ other) / Z with Z from a ones-matmul.
"""

import os
import sys

sys.path.insert(0, "/opt/trn_rl_repo")

import numpy as np

from concourse import bacc, mybir, tile
from concourse.bass_utils import run_bass_kernel_spmd

B, S, H, K = 32, 128, 512, 8
NCORES = 8
BL = B // NCORES          # batches per core
BS = BL * S               # rows per core (512)
P = 128                   # partitions
HT = H // P               # h partition-tiles (4)
KG = K * H                # flattened (k,g) axis (4096)
KC = K // 2               # k's per stage-2 chunk (4)
F32 = mybir.dt.float32
F32R = mybir.dt.float32r
F16 = mybir.dt.float16

_cache = {}


def _build():
    """Build + compile the per-core Bass program (same program on all cores)."""
    nc = bacc.Bacc("TRN2", target_bir_lowering=False, debug=False,
                   num_devices=NCORES)

    # head images: [xa_h0|xa_h1|g_k0h0|g_k0h1] and the h2/h3 twin (fp16)
    head_a_d = nc.dram_tensor("head_a", [P, 2 * BS + 2 * H], F16,
                              kind="ExternalInput")
    head_b_d = nc.dram_tensor("head_b", [P, 2 * BS + 2 * H], F16,
                              kind="ExternalInput")
    xp_t_d = nc.dram_tensor("xp_t", [P, HT * BS], F16, kind="ExternalInput")
    xa_nat_d = nc.dram_tensor("xa_nat", [P, BL * H], F16, kind="ExternalInput")
    xp_nat_d = nc.dram_tensor("xp_nat", [P, BL * H], F16, kind="ExternalInput")
    # G: one pre-shuffled tensor per branch; column blocks in consumption
    # order: k0 h-major block (4H), k1 h-major block (4H), then per-k blocks
    g_ap_d = nc.dram_tensor("g_ap", [P, HT * KG], F16, kind="ExternalInput")
    g_pa_d = nc.dram_tensor("g_pa", [P, HT * KG], F16, kind="ExternalInput")
    v_ap_d = nc.dram_tensor("v_ap", [K, 1], F32, kind="ExternalInput")
    v_pa_d = nc.dram_tensor("v_pa", [K, 1], F32, kind="ExternalInput")
    out_a_d = nc.dram_tensor("out_a", [BS, H], F16, kind="ExternalOutput")
    out_p_d = nc.dram_tensor("out_p", [BS, H], F16, kind="ExternalOutput")

    Tanh = mybir.ActivationFunctionType.Tanh
    Exp = mybir.ActivationFunctionType.Exp
    MULT = mybir.AluOpType.mult
    ADD = mybir.AluOpType.add

    with tile.TileContext(nc) as tc:
        with (
            tc.tile_pool(name="const", bufs=1) as cpool,
            tc.tile_pool(name="g", bufs=1) as gpool,
            tc.tile_pool(name="proj", bufs=1) as projpool,
            tc.tile_pool(name="work", bufs=2) as work,
            tc.tile_pool(name="ps_acc", bufs=5, space="PSUM") as ps_acc,
            tc.tile_pool(name="ps_o", bufs=2, space="PSUM") as ps_o,
            tc.tile_pool(name="ps_z", bufs=1, space="PSUM") as ps_z,
        ):
            # ---- constants + warmup weights --------------------------------
            wm = cpool.tile([P, BS], F32R, tag="wm")
            nc.vector.memset(wm[:].bitcast(F32), 0.0)
            ones_t = cpool.tile([P, 2], F16, tag="ones_t")
            nc.vector.memset(ones_t[:], 1.0)

            # ---- persistent activations ------------------------------------
            head_a = cpool.tile([P, 2 * BS + 2 * H], F16, tag="head_a")
            head_b = cpool.tile([P, 2 * BS + 2 * H], F16, tag="head_b")
            xp_t = cpool.tile([P, HT * BS], F16, tag="xp_t")
            xa_nat = cpool.tile([P, BL * H], F16, tag="xa_nat")
            xp_nat = cpool.tile([P, BL * H], F16, tag="xp_nat")

            vrow_a = cpool.tile([1, K], F32, tag="vrow_a")
            vrow_p = cpool.tile([1, K], F32, tag="vrow_p")
            vbc_a = cpool.tile([P, K], F32, tag="vbc_a")
            vbc_p = cpool.tile([P, K], F32, tag="vbc_p")

            # ---- G piece tiles (bufs=1; k0/k1 recycled for branch 1) -------
            g_k0 = gpool.tile([P, HT * H], F16, tag="g_k0")
            g_k1 = gpool.tile([P, HT * H], F16, tag="g_k1")
            g_kk = [gpool.tile([P, HT * H], F16, tag=f"g_kk{i}",
                               name=f"g_kk{i}")
                    for i in range(6)]

            def load_g_k(g_d, i):
                o = (2 + i) * HT * H
                nc.sync.dma_start(out=g_kk[i][:], in_=g_d.ap()[:, o:o + HT * H])

            # ---- startup loads, consumption order. The first k1/k2 pieces
            # ride the gpsimd/scalar/vector queues so their issue+transfer
            # overlaps the sync queue's head stream (issues serialize ~1us
            # per dma_start within one queue).
            nc.sync.dma_start(out=head_a[:], in_=head_a_d.ap()[:])
            nc.sync.dma_start(out=head_b[:], in_=head_b_d.ap()[:])
            nc.gpsimd.dma_start(out=vrow_a[:], in_=v_ap_d.ap().rearrange("k o -> o k"))
            nc.gpsimd.dma_start(out=vrow_p[:], in_=v_pa_d.ap().rearrange("k o -> o k"))
            nc.gpsimd.partition_broadcast(vbc_a[:], vrow_a[:])
            nc.gpsimd.partition_broadcast(vbc_p[:], vrow_p[:])
            nc.sync.dma_start(out=g_k1[:], in_=g_ap_d.ap()[:, 4 * H:8 * H])
            for i in range(4):
                load_g_k(g_ap_d, i)
            nc.sync.dma_start(out=xp_t[:], in_=xp_t_d.ap()[:])
            load_g_k(g_ap_d, 4)
            load_g_k(g_ap_d, 5)
            nc.sync.dma_start(out=xp_nat[:], in_=xp_nat_d.ap()[:])
            # (g_pa loads are emitted after br0-s1 so the WAR deps pick up
            #  br0's reads; xa_nat after those.)

            def xa_rhs(h):
                t = head_a if h < 2 else head_b
                return t[:, (h % 2) * BS:(h % 2 + 1) * BS]

            def xa_lhsT(gi, b):
                t = head_a if gi < 2 else head_b
                o = (gi % 2) * BS + b * S
                return t[:, o:o + S]

            def xp_lhsT(gi, b):
                o = gi * BS + b * S
                return xp_t[:, o:o + S]

            def g0_head(h, gt):
                # branch-0 k0 weights live in the head images
                t = head_a if h < 2 else head_b
                o = 2 * BS + (h % 2) * H + gt * P
                return t[:, o:o + P]

            def g0_tile(h, gt):
                return g_k0[:, h * H + gt * P:h * H + gt * P + P]

            def g_lhsT(k, h, gt, g0_at):
                if k == 0:
                    return g0_at(h, gt)
                piece = g_k1 if k == 1 else g_kk[k - 2]
                o = h * H + gt * P
                return piece[:, o:o + P]

            # projT2[br][gt][g_part, k, b, s] — stage-1 output, stage-2 rhs;
            # per-branch buffers so br1's evacs never WAR-wait on br0-s2.
            projT2 = [[projpool.tile([P, K, BL, S], F16,
                                     tag=f"projT2_{br}_{gt}",
                                     name=f"projT2_{br}_{gt}")
                       for gt in range(HT)]
                      for br in range(2)]

            evac_state = [0]

            def evac(dst, src):
                # ping-pong PSUM evacuations between vector and scalar
                if evac_state[0] % 2 == 0:
                    nc.vector.tensor_copy(dst, src)
                else:
                    nc.scalar.copy(dst, src)
                evac_state[0] += 1

            # ---- PE warmup: get the p-state ramp going during DMA lead-in
            for w in range(6):
                acc = ps_acc.tile([P, BL, S], F32, tag="acc", name=f"warm{w}")
                nc.tensor.matmul(acc[:], wm[:, 0:P], wm[:],
                                 start=True, stop=True)

            def filler(name):
                acc = ps_acc.tile([P, BL, S], F32, tag="acc", name=name)
                nc.tensor.matmul(acc[:], wm[:, 0:P], wm[:],
                                 start=True, stop=True)

            def stage1(br, g0_at):
                # k0 h-outer with 4 open accumulators: first matmuls need
                # only the head_a image.
                accs = [ps_acc.tile([P, BL, S], F32, tag="acc",
                                    name=f"s1a{br}k0g{gt}")
                        for gt in range(HT)]
                for h in range(HT):
                    for gt in range(HT):
                        nc.tensor.matmul(
                            accs[gt][:], g_lhsT(0, h, gt, g0_at), xa_rhs(h),
                            start=(h == 0), stop=(h == HT - 1),
                            skip_group_check=True)
                for gt in range(HT):
                    evac(projT2[br][gt][:, 0, :, :], accs[gt][:])
                for k in range(1, K):
                    for gt in range(HT):
                        acc = ps_acc.tile([P, BL, S], F32, tag="acc",
                                          name=f"s1a{br}k{k}g{gt}")
                        for h in range(HT):
                            nc.tensor.matmul(
                                acc[:], g_lhsT(k, h, gt, g0_at), xa_rhs(h),
                                start=(h == 0), stop=(h == HT - 1))
                        evac(projT2[br][gt][:, k, :, :], acc[:])
                    yield k

            # th_all[t_part, k, b, s]: tanh(stage-2) output, both branches
            # (WAR-recycled). Score ops slice [:, j, :, :] batched over b.
            th_all = work.tile([P, K, BL, S], F16, tag="th", bufs=1)

            def stage2_ck(br, lhsT_of, klo, nk, bs=range(BL)):
                # raw[t, k, s] = tanh(sum_g right[t,g] proj[g,k,s]) per batch
                for b in bs:
                    acc2 = ps_acc.tile([P, nk, S], F32, tag="acc",
                                       name=f"s2a{br}b{b}c{klo}")
                    for gi in range(HT):
                        nc.tensor.matmul(
                            acc2[:],
                            lhsT_of(gi, b),
                            projT2[br][gi][:, klo:klo + nk, b, :],
                            start=(gi == 0), stop=(gi == HT - 1))
                    nc.scalar.activation(
                        th_all[:, klo:klo + nk, b, :], acc2[:], Tanh)

            def sca_all(vbc):
                # first-half score partial, batched over all 4 batches
                sca = work.tile([P, BL, S], F16, tag="sca", bufs=1)
                nc.vector.tensor_scalar_mul(sca[:], th_all[:, 0, :, :],
                                            vbc[:, 0:1])
                for j in range(1, KC):
                    nc.vector.scalar_tensor_tensor(
                        sca[:], th_all[:, j, :, :], vbc[:, j:j + 1], sca[:],
                        MULT, ADD)
                return sca

            def zout(br, b, e_t, nat_other, nat_self, out_d):
                # out = self + (E_T.T @ other) / Z, Z via ones-matmul.
                zp = ps_z.tile([P, 2], F32, tag="z", name=f"z{br}b{b}")
                nc.tensor.matmul(zp[:], e_t[:, b, :], ones_t[:],
                                 start=True, stop=True)
                rz = work.tile([P, 1], F32, tag="rz", bufs=4)
                nc.vector.reciprocal(rz[:], zp[:, 0:1])
                rp = ps_o.tile([P, H], F32, tag="o", name=f"o{br}b{b}")
                nc.tensor.matmul(rp[:], e_t[:, b, :], nat_other[:, b * H:(b + 1) * H],
                                 start=True, stop=True)
                ot = work.tile([P, H], F16, tag="ot", bufs=4)
                nc.vector.scalar_tensor_tensor(
                    ot[:], rp[:], rz[:, 0:1], nat_self[:, b * H:(b + 1) * H],
                    MULT, ADD)
                nc.sync.dma_start(out=out_d.ap()[b * P:(b + 1) * P, :],
                                  in_=ot[:])

            e_t0 = work.tile([P, BL, S], F16, tag="e0", bufs=1)
            e_t1 = work.tile([P, BL, S], F16, tag="e1", bufs=1)

            # ================= branch 0 (aspect) ========================
            for k in stage1(0, g0_head):
                if k == 5:
                    stage2_ck(0, xp_lhsT, 0, KC)
                    sca0 = sca_all(vbc_a)
            # br1 G prefetch; WAR-gated on br0-s1 reads
            nc.sync.dma_start(out=g_k0[:], in_=g_pa_d.ap()[:, 0:4 * H])
            nc.sync.dma_start(out=g_k1[:], in_=g_pa_d.ap()[:, 4 * H:8 * H])
            nc.sync.dma_start(out=xa_nat[:], in_=xa_nat_d.ap()[:])
            for i in range(6):
                load_g_k(g_pa_d, i)
            stage2_ck(0, xp_lhsT, KC, KC)
            # batched second half + exp (runs under br1-s1)
            scb0 = work.tile([P, BL, S], F16, tag="scb", bufs=1)
            nc.vector.tensor_scalar_mul(scb0[:], th_all[:, KC, :, :],
                                        vbc_a[:, KC:KC + 1])
            for j in range(1, KC):
                nc.vector.scalar_tensor_tensor(
                    scb0[:], th_all[:, KC + j, :, :],
                    vbc_a[:, KC + j:KC + j + 1], scb0[:], MULT, ADD)
            nc.vector.tensor_tensor(scb0[:], sca0[:], scb0[:], ADD)
            nc.scalar.activation(e_t0[:], scb0[:], Exp)

            # ================= branch 1 (polarity) ======================
            # br1 stage 1 with br1-s2-ck0 spliced at k=5 and br0's z/out
            # matmuls interleaved so the PE never waits on softmax chains.
            sca1b = work.tile([P, BL, S], F16, tag="sca1b", bufs=1)
            zo = 0
            for k in stage1(1, g0_tile):
                if k == 5:
                    stage2_ck(1, xa_lhsT, 0, KC)
                    sca1 = sca_all(vbc_p)
                elif k == 6:
                    # k4,k5 stage-2 spliced here so the tail chunk is k6,k7
                    stage2_ck(1, xa_lhsT, KC, 2)
                elif k == 7:
                    # fold k4,k5 into the score partial (batched, on vector)
                    nc.vector.scalar_tensor_tensor(
                        sca1b[:], th_all[:, KC, :, :], vbc_p[:, KC:KC + 1],
                        sca1[:], MULT, ADD)
                    nc.vector.scalar_tensor_tensor(
                        sca1b[:], th_all[:, KC + 1, :, :],
                        vbc_p[:, KC + 1:KC + 2], sca1b[:], MULT, ADD)
                    zout(0, zo, e_t0, xp_nat, xa_nat, out_a_d)
                    zo += 1
                elif k in (2, 3, 4):
                    zout(0, zo, e_t0, xp_nat, xa_nat, out_a_d)
                    zo += 1

            # ---- tail: k6,k7 pair-at-a-time, 2-op fp16 vector chain + exp -
            def chain_pair(pr):
                bs2 = slice(2 * pr, 2 * pr + 2)
                cha = work.tile([P, 2, S], F16, tag=f"cha{pr}", bufs=1)
                nc.vector.scalar_tensor_tensor(
                    cha[:], th_all[:, KC + 2, bs2, :], vbc_p[:, KC + 2:KC + 3],
                    sca1b[:, bs2, :], MULT, ADD)
                nc.vector.scalar_tensor_tensor(
                    cha[:], th_all[:, KC + 3, bs2, :], vbc_p[:, KC + 3:KC + 4],
                    cha[:], MULT, ADD)
                nc.scalar.activation(e_t1[:, bs2, :], cha[:], Exp)
                return cha

            Copy = mybir.ActivationFunctionType.Copy

            def ztail(b, on_vector):
                zp = ps_z.tile([P, 2], F32, tag="z", name=f"z1b{b}")
                nc.tensor.matmul(zp[:], e_t1[:, b, :], ones_t[:],
                                 start=True, stop=True)
                rz = work.tile([P, 1], F32, tag="rz", bufs=4)
                nc.vector.reciprocal(rz[:], zp[:, 0:1])
                rp = ps_o.tile([P, H], F32, tag="o", name=f"o1b{b}")
                nc.tensor.matmul(rp[:], e_t1[:, b, :], xa_nat[:, b * H:(b + 1) * H],
                                 start=True, stop=True)
                if on_vector:
                    otf = work.tile([P, H], F16, tag=f"otf{b}", bufs=1)
                    nc.vector.scalar_tensor_tensor(
                        otf[:], rp[:], rz[:, 0:1], xp_nat[:, b * H:(b + 1) * H],
                        MULT, ADD)
                else:
                    # keep vector free for the pair-1 chain: scale on the
                    # act engine, residual-add on gpsimd (slack batches)
                    ots = work.tile([P, H], F32, tag=f"ots{b}", bufs=1)
                    nc.scalar.activation(ots[:], rp[:], Copy, scale=rz[:, 0:1])
                    otf = work.tile([P, H], F16, tag=f"otf{b}", bufs=1)
                    nc.gpsimd.tensor_tensor(
                        otf[:], ots[:], xp_nat[:, b * H:(b + 1) * H], ADD)
                nc.sync.dma_start(out=out_p_d.ap()[b * P:(b + 1) * P, :],
                                  in_=otf[:])

            def filler_dep(name, lhsT, rhs):
                # PE keep-alive matmul gated on a real dependency so the
                # tile scheduler cannot hoist it ahead of the wait
                acc = ps_acc.tile([P, BL, S], F32, tag="acc", name=name)
                nc.tensor.matmul(acc[:], lhsT, rhs, start=True, stop=True)

            stage2_ck(1, xa_lhsT, KC + 2, 2, bs=(0, 1))
            cha0 = chain_pair(0)              # under b2/b3 matmuls
            stage2_ck(1, xa_lhsT, KC + 2, 2, bs=(2, 3))
            # PE keep-alives bridging the exp latencies (gated so the
            # scheduler cannot hoist them before the waits they cover)
            filler_dep("f0", th_all[:, KC + 2, 1, :],
                       th_all[:, KC + 2:KC + 4, 0:2, :])
            filler_dep("f0b", th_all[:, KC + 3, 1, :],
                       th_all[:, KC + 2:KC + 4, 2:4, :])
            filler_dep("f1", cha0[:, 0, :], th_all[:, KC:2 * KC, 0, :])
            filler_dep("f1b", cha0[:, 1, :], th_all[:, KC:2 * KC, 1, :])
            cha1 = chain_pair(1)              # vector, right after tanh b3
            ztail(0, on_vector=False)
            ztail(1, on_vector=False)
            filler_dep("f2", cha1[:, 0, :], th_all[:, KC:2 * KC, 2, :])
            filler_dep("f3", cha1[:, 1, :], th_all[:, KC:2 * KC, 3, :])
            ztail(2, on_vector=True)
            ztail(3, on_vector=True)

    nc.compile()
    return nc


def _get_nc():
    if "nc" not in _cache:
        _cache["nc"] = _build()
    return _cache["nc"]


def _prep_in_maps(aspect_hidden, polarity_hidden, G_aspect_polarity,
                  G_polarity_aspect, G_vector_aspect, G_vector_polarity):
    f16 = np.float16

    def shuffle_g(g):
        # host-side image of the SBUF G tiles, concatenated in consumption
        # order: k0 h-major block, k1 h-major block, then per-k blocks
        gr = np.asarray(g, f16).reshape(HT, P, K, H)
        pieces = [gr[:, :, k, :].transpose(1, 0, 2).reshape(P, HT * H)
                  for k in range(K)]
        return np.ascontiguousarray(np.concatenate(pieces, axis=1))

    def shuffle_t(x_loc):
        # [BS,H] -> transposed partition-major [P, (ht, bs)]
        return np.ascontiguousarray(
            x_loc.T.reshape(HT, P, BS).transpose(1, 0, 2).reshape(P, HT * BS))

    def shuffle_nat(x_loc):
        # [BS,H] -> partition-major [P, (b, h)]
        return np.ascontiguousarray(
            x_loc.reshape(BL, P, H).transpose(1, 0, 2).reshape(P, BL * H))

    a = np.asarray(aspect_hidden, f16)
    p = np.asarray(polarity_hidden, f16)
    g_ap = shuffle_g(G_aspect_polarity)
    g_pa = shuffle_g(G_polarity_aspect)
    v_ap = np.ascontiguousarray(G_vector_aspect, np.float32)
    v_pa = np.ascontiguousarray(G_vector_polarity, np.float32)

    in_maps = []
    for c in range(NCORES):
        a_loc = a[c * BL:(c + 1) * BL].reshape(BS, H)
        p_loc = p[c * BL:(c + 1) * BL].reshape(BS, H)
        xa_t = shuffle_t(a_loc)
        m = {
            "head_a": np.ascontiguousarray(
                np.concatenate([xa_t[:, 0:2 * BS], g_ap[:, 0:2 * H]], axis=1)),
            "head_b": np.ascontiguousarray(
                np.concatenate([xa_t[:, 2 * BS:], g_ap[:, 2 * H:4 * H]], axis=1)),
            "xp_t": shuffle_t(p_loc),
            "xa_nat": shuffle_nat(a_loc),
            "xp_nat": shuffle_nat(p_loc),
            "g_ap": g_ap,
            "g_pa": g_pa,
            "v_ap": v_ap,
            "v_pa": v_pa,
        }
        in_maps.append(m)
    return in_maps


def kernel(aspect_hidden, polarity_hidden, G_aspect_polarity,
           G_polarity_aspect, G_vector_aspect, G_vector_polarity):
    nc = _get_nc()
    in_maps = _prep_in_maps(aspect_hidden, polarity_hidden, G_aspect_polarity,
                            G_polarity_aspect, G_vector_aspect,
                            G_vector_polarity)
    res = run_bass_kernel_spmd(
        nc, in_maps, core_ids=list(range(NCORES)),
        trace=bool(os.environ.get("KERNEL_TRACE")))
    _cache["last_results"] = res

    out_a = np.empty((B, S, H), np.float32)
    out_p = np.empty((B, S, H), np.float32)
    for c in range(NCORES):
        out_a[c * BL:(c + 1) * BL] = res.results[c]["out_a"].astype(
            np.float32).reshape(BL, S, H)
        out_p[c * BL:(c + 1) * BL] = res.results[c]["out_p"].astype(
            np.float32).reshape(BL, S, H)
    return (out_a, out_p)
